# revision 18
# baseline (speedup 1.0000x reference)
"""Masked-softmax attention on 8 trn2 NeuronCores.

Reference computation (per batch b):
    att = q @ k                        # [n_q, n_k], k given pre-transposed [d, n_k]
    att = where(mask==0, -1e9, att)
    att = softmax(att, -1) / sqrt(d)
    out = (att @ v).T                  # returned [n_dv, n_q]

Sharding: data-parallel over batch: B=16 -> 2 batches per core x 8 cores.

Host-side, per batch, the key dimension is COMPACTED: masked-out keys
contribute exactly 0 to both the softmax numerator and denominator (the
reference's exp(-1e9 - anything) underflows to +0.0 in fp32), so we gather
only the unmasked columns of k / rows of v, padded up to a multiple of 128
(padding killed by the same -1e9 bias). With a Bernoulli(0.5) mask this
halves the contraction length. Exact, not an approximation.

q and k are cast to FP16 host-side (10-bit mantissa; adds ~1.2e-3 rel err
vs the 2e-2 gate, measured): halves the startup-critical DMA bytes and
runs the QK^T matmuls at full PE rate with fast weight loads.

Device-side plan (per batch, in the TRANSPOSED score layout S^T[k, q]):
    - S^T tile [128k, 512q] = k_slice[d,128k]^T @ qT[d, 512q]  (fp16,
      2 d-half accumulation steps)
    - softmax is shift-invariant: subtract a CONSTANT shift (scores ~ N(0, d)
      with d=256 -> |s| < ~110 always). Mask + shift fold into the ACT-engine
      exp as a per-partition bias: e = exp(s + bias_k), in f32r.
    - out^T[dv, q] += v_tile[128k, dv_chunk]^T @ e   (f32r, full rate)
    - Z: e-tiles are pairwise-accumulated on DVE in groups of QUAD; each
      group closes with a matmul against an all-16s stationary that row-sums
      AND broadcasts 16Z to every partition (folding the post-softmax
      1/sqrt(d)=1/16 scale) into one PSUM bank.
    - out = out^T * (1/(16Z)) via DVE approx-reciprocal + multiply.

DMA schedule (the aggregate HBM bandwidth ~358GB/s is the startup
constraint, so only stripe-0-critical bytes move first):
    - gpsimd SWDGE: q stripe-0 halves + batch-0 k tiles interleaved (the
      first matmul needs q00h0 + k0 only), then the remaining q stripes.
    - sync HWDGE: batch-0 v tiles, then ALL batch-1 tiles (prefetch), then
      o0 stores.
    - scalar HWDGE (= ACT engine): bias only, before the exp stream starts;
      o1 stores ride along later (~0.6us each, inside ACT's slack).
Dep-free fp32 warmup matmuls bridge the PE from ring-open (~7.2us, fixed
NEFF preamble) to the first real matmul so the HAM clock gate reaches
K=8/8 without re-throttling mid-kernel.
"""

from contextlib import ExitStack

import numpy as np

import concourse.bacc as bacc
import concourse.mybir as mybir
import concourse.tile as tile
from concourse.bass_utils import run_bass_kernel_spmd

P = 128          # partitions
D = 256          # d == n_dv
S = 2048         # n_q
NB = 2           # batches per core
QS = 512         # q-stripe width (max fp32 matmul moving dim)
NQS = S // QS    # 4 q-stripes
NCORES = 8
SHIFT = 60.0     # constant softmax shift (see module docstring)

F32 = mybir.dt.float32
F32R = mybir.dt.float32r
F16 = mybir.dt.float16
EXP = mybir.ActivationFunctionType.Exp
MULT = mybir.AluOpType.mult
ADD = mybir.AluOpType.add

N_WARM_MID = 2    # fp32 [128,256] warmups (2 half-rate passes each)
N_WARM_SMALL = 2  # fp32 [128,128] warmups
QUAD = 5          # e-tiles accumulated on DVE per Z matmul


def build(nkt):
    """Build the per-core program. nkt = compacted key length / 128."""
    nc = bacc.Bacc()
    kk = nc.declare_dram_parameter("k", [NB, nkt, P, 2 * P], F16, isOutput=False)
    vv = nc.declare_dram_parameter("v", [NB, nkt, P, 2 * P], F32R, isOutput=False)
    qq = nc.declare_dram_parameter("q", [NB, NQS, P, 2 * QS], F16, isOutput=False)
    bb = nc.declare_dram_parameter("bias", [NB, P, nkt], F32, isOutput=False)
    out = nc.declare_dram_parameter("out", [NB, D, S], F32, isOutput=True)

    with tile.TileContext(nc) as tc, ExitStack() as ctx:
        consts = ctx.enter_context(tc.tile_pool(name="consts", bufs=1))
        inp = ctx.enter_context(tc.tile_pool(name="inp", bufs=1))
        epool = ctx.enter_context(tc.tile_pool(name="e", bufs=6))
        apool = ctx.enter_context(tc.tile_pool(name="a", bufs=3))
        opool = ctx.enter_context(tc.tile_pool(name="o", bufs=2))
        zpool = ctx.enter_context(tc.tile_pool(name="z", bufs=2))
        ps_s = ctx.enter_context(tc.tile_pool(name="ps_s", bufs=3, space="PSUM"))
        ps_o = ctx.enter_context(tc.tile_pool(name="ps_o", bufs=2, space="PSUM"))
        ps_z = ctx.enter_context(tc.tile_pool(name="ps_z", bufs=1, space="PSUM"))

        # ---- constants; memsets on gpsimd (its queue opens first, ~6.0us)
        warm_f = consts.tile([P, P], F32)
        nc.gpsimd.memset(warm_f, 1.0)
        warm_w = consts.tile([P, 2 * P], F32)
        nc.gpsimd.memset(warm_w, 0.5)
        sixteens_f = consts.tile([P, P], F32)
        nc.gpsimd.memset(sixteens_f, 16.0)
        # (memset can't emit f32r; DVE-copy to round)
        sixteens = consts.tile([P, P], F32R)
        nc.vector.tensor_copy(sixteens, sixteens_f)

        # Warmup Exp: attaches the implicit ~2.7us ACT table load to a
        # dep-light instruction so it overlaps the input-DMA fill.
        warm_out = consts.tile([P, 1], F32)
        nc.scalar.activation(warm_out, warm_f[:, 0:1], EXP)

        # PE warmups: dep-free matmuls from ring-open until real matmuls
        # flow, so the HAM clock gate reaches K=8/8 with no re-throttle.
        for w in range(N_WARM_MID):
            wp = ps_s.tile([P, QS], F32, tag="s", name=f"warmb{w}")
            nc.tensor.matmul(
                wp[:, 0 : 2 * P], lhsT=warm_f, rhs=warm_w, start=True, stop=True
            )
        for w in range(N_WARM_SMALL):
            wp = ps_s.tile([P, QS], F32, tag="s", name=f"warms{w}")
            nc.tensor.matmul(
                wp[:, 0:P], lhsT=warm_f, rhs=warm_f[:, 0:P], start=True, stop=True
            )

        # ---- input DMAs, issued upfront in consumption order
        # bias rides sync first (tiny); keeps the ACT ring free so the exp
        # stream can start the moment the first scores land
        biast = []
        for b in range(NB):
            bt = inp.tile([P, nkt], F32, tag=f"bias{b}")
            nc.sync.dma_start(out=bt, in_=bb[b])
            biast.append(bt)
        # gpsimd ring: q00 halves + batch-0 k tiles interleaved (the first
        # matmul needs q00h0 + k0 only), then the remaining q stripes
        kts = [[], []]
        q00 = [None, None]
        qts = [[None] * NQS for _ in range(NB)]
        q00[0] = inp.tile([P, QS], F16, tag="q00_0", name="q00_0")
        nc.gpsimd.dma_start(out=q00[0], in_=qq[0, 0, :, 0:QS])
        kt0 = inp.tile([P, 2 * P], F16, tag="k0_0", name="kt0")
        nc.gpsimd.dma_start(out=kt0, in_=kk[0, 0])
        kts[0].append(kt0)
        q00[1] = inp.tile([P, QS], F16, tag="q00_1", name="q00_1")
        nc.gpsimd.dma_start(out=q00[1], in_=qq[0, 0, :, QS : 2 * QS])
        for t in range(1, nkt):
            kt = inp.tile([P, 2 * P], F16, tag=f"k0_{t}")
            nc.gpsimd.dma_start(out=kt, in_=kk[0, t])
            kts[0].append(kt)
        for b in range(NB):
            for s in range(NQS):
                if b == 0 and s == 0:
                    continue
                qt = inp.tile([P, 2 * QS], F16, tag=f"q{b}_{s}")
                nc.gpsimd.dma_start(out=qt, in_=qq[b, s])
                qts[b][s] = qt
        # sync ring: batch-0 v tiles per-tile (streamed under stripe 0),
        # then all of batch 1 in 2 blocks per tensor (prefetch)
        vts = [[], []]
        for t in range(nkt):
            vt = inp.tile([P, 2 * P], F32R, tag=f"v0_{t}")
            nc.sync.dma_start(out=vt, in_=vv[0, t])
            vts[0].append(vt)
        h = (nkt + 1) // 2
        for prefix, param, dt, row in (("k1", kk, F16, kts[1]), ("v1", vv, F32R, vts[1])):
            for t0, t1 in ((0, h), (h, nkt)):
                blk = inp.tile(
                    [P, t1 - t0, 2 * P], dt, tag=f"{prefix}_{t0}", name=f"{prefix}_{t0}"
                )
                nc.sync.dma_start(
                    out=blk, in_=param[1, t0:t1].rearrange("t p c -> p t c")
                )
                row.extend(blk[:, t - t0, :] for t in range(t0, t1))

        # ---- compute, one 512-wide q-stripe at a time
        for b in range(NB):
            for s in range(NQS):
                if b == 0 and s == 0:
                    qh = (q00[0], q00[1])
                else:
                    qt = qts[b][s]
                    qh = (qt[:, 0:QS], qt[:, QS : 2 * QS])
                op0 = ps_o.tile([P, QS], F32, tag="o0", name="op0")
                op1 = ps_o.tile([P, QS], F32, tag="o1", name="op1")
                zp = ps_z.tile([P, QS], F32, tag="z", name="zp")
                nzmm = (nkt + QUAD - 1) // QUAD
                acc, nacc, zi = None, 0, 0
                for t in range(nkt):
                    kt, vt = kts[b][t], vts[b][t]
                    sp = ps_s.tile([P, QS], F32, tag="s", name="sp")
                    nc.tensor.matmul(
                        sp, lhsT=kt[:, 0:P], rhs=qh[0], start=True, stop=False
                    )
                    nc.tensor.matmul(
                        sp, lhsT=kt[:, P : 2 * P], rhs=qh[1], start=False, stop=True
                    )
                    e = epool.tile([P, QS], F32R, tag="e", name="e")
                    nc.scalar.activation(e, sp, EXP, bias=biast[b][:, t : t + 1])
                    first, last = t == 0, t == nkt - 1
                    nc.tensor.matmul(
                        op0, lhsT=vt[:, 0:P], rhs=e, start=first, stop=last
                    )
                    nc.tensor.matmul(
                        op1, lhsT=vt[:, P : 2 * P], rhs=e, start=first, stop=last
                    )
                    # running DVE accumulator for the softmax denominator;
                    # every QUAD tiles one Z matmul folds the partial into
                    # PSUM (row-sum + broadcast of 16Z to every partition,
                    # folding the post-softmax 1/sqrt(d)=1/16 scale)
                    if acc is None:
                        acc, nacc = e, 1
                    else:
                        na = apool.tile([P, QS], F32R, tag="a", name="na")
                        nc.vector.tensor_tensor(na, acc, e, ADD)
                        acc, nacc = na, nacc + 1
                    if nacc == QUAD or t == nkt - 1:
                        nc.tensor.matmul(
                            zp, lhsT=sixteens, rhs=acc,
                            start=zi == 0, stop=zi == nzmm - 1,
                        )
                        zi += 1
                        acc, nacc = None, 0

                # normalize: out = out_unnorm * (1/(16Z)). ~18-bit approx
                # reciprocal; z is far from denorm/inf so edge cases can't
                # hit. Last stripe runs in halves so the tail pipelines.
                halves = 4 if (b == NB - 1 and s == NQS - 1) else 1
                hw = QS // halves
                zbs = zpool.tile([P, QS], F32, tag="zbs", name="zbs")
                o0 = opool.tile([P, QS], F32, tag="so0", name="o0")
                o1 = opool.tile([P, QS], F32, tag="so1", name="o1")
                for h in range(halves):
                    hs = slice(h * hw, (h + 1) * hw)
                    osl = slice(s * QS + h * hw, s * QS + (h + 1) * hw)
                    nc.vector.reciprocal_approx_fast(out=zbs[:, hs], in_=zp[:, hs])
                    nc.vector.tensor_tensor(o0[:, hs], op0[:, hs], zbs[:, hs], MULT)
                    nc.vector.tensor_tensor(o1[:, hs], op1[:, hs], zbs[:, hs], MULT)
                    nc.sync.dma_start(out=out[b, 0:P, osl], in_=o0[:, hs])
                    nc.scalar.dma_start(out=out[b, P : 2 * P, osl], in_=o1[:, hs])

    return nc


def make_in_maps(q, k, v, mask):
    """Shard over batch; tile + compact; cast q,k to fp16."""
    q = np.asarray(q, dtype=np.float32)
    k = np.asarray(k, dtype=np.float32)
    v = np.asarray(v, dtype=np.float32)
    mask = np.asarray(mask, dtype=np.int32).reshape(len(q), -1)

    B = len(q)
    idxs = [np.nonzero(mask[b])[0] for b in range(B)]
    n_eff = max((len(ix) for ix in idxs), default=1)
    sk = max(P, ((n_eff + P - 1) // P) * P)  # padded compacted key length
    nkt = sk // P

    kg = np.zeros((B, D, sk), dtype=np.float32)
    vg = np.zeros((B, sk, D), dtype=np.float32)
    # exp bias: -SHIFT for real keys, -1e9 for padding (kills it exactly)
    bg = np.full((B, sk), -1.0e9, dtype=np.float32)
    for b in range(B):
        ix = idxs[b]
        kg[b, :, : len(ix)] = k[b][:, ix]
        vg[b, : len(ix)] = v[b][ix]
        bg[b, : len(ix)] = -SHIFT

    # k tiles: [B, nkt, P, 256] fp16: cols 0:128 = d-half0 keys, 128:256 = half1
    kh = np.ascontiguousarray(
        kg.reshape(B, 2, P, nkt, P).transpose(0, 3, 2, 1, 4).reshape(B, nkt, P, 2 * P)
    ).astype(np.float16)
    # v tiles: [B, nkt, P, 256] f32: partition = key-within-tile, cols = dv
    vh = np.ascontiguousarray(vg.reshape(B, nkt, P, 2 * P))
    # q stripes: [B, NQS, P, 1024] fp16 = both d-halves of qT side by side
    qt = np.transpose(q, (0, 2, 1))  # [B, D, S]
    qh = np.ascontiguousarray(
        qt.reshape(B, 2, P, NQS, QS).transpose(0, 3, 2, 1, 4).reshape(B, NQS, P, 2 * QS)
    ).astype(np.float16)
    bgt = np.ascontiguousarray(bg.reshape(B, nkt, P).transpose(0, 2, 1))  # [B, P, nkt]

    in_maps = []
    for i in range(NCORES):
        sl = slice(i * NB, (i + 1) * NB)
        in_maps.append(
            {
                "k": np.ascontiguousarray(kh[sl]),
                "v": np.ascontiguousarray(vh[sl]),
                "q": np.ascontiguousarray(qh[sl]),
                "bias": np.ascontiguousarray(bgt[sl]),
            }
        )
    return in_maps, nkt


def run(q, k, v, mask, **kwargs):
    in_maps, nkt = make_in_maps(q, k, v, mask)
    nc = build(nkt)
    nc.finalize()  # run the Bacc pass pipeline (reg alloc, wait splitting)
    res = run_bass_kernel_spmd(nc, in_maps, list(range(NCORES)), **kwargs)
    out = np.concatenate([r["out"] for r in res.results], axis=0)
    return out, res


def kernel(q, k, v, mask):
    out, _ = run(q, k, v, mask)
    return out


# revision 19
# speedup vs baseline: 1.0112x; 1.0112x over previous
"""Masked-softmax attention on 8 trn2 NeuronCores.

Reference computation (per batch b):
    att = q @ k                        # [n_q, n_k], k given pre-transposed [d, n_k]
    att = where(mask==0, -1e9, att)
    att = softmax(att, -1) / sqrt(d)
    out = (att @ v).T                  # returned [n_dv, n_q]

Sharding: data-parallel over batch: B=16 -> 2 batches per core x 8 cores.

Host-side, per batch, the key dimension is COMPACTED: masked-out keys
contribute exactly 0 to both the softmax numerator and denominator (the
reference's exp(-1e9 - anything) underflows to +0.0 in fp32), so we gather
only the unmasked columns of k / rows of v, padded up to a multiple of 128
(padding killed by the same -1e9 bias). With a Bernoulli(0.5) mask this
halves the contraction length. Exact, not an approximation.

q and k are cast to FP16 host-side (10-bit mantissa; adds ~1.2e-3 rel err
vs the 2e-2 gate, measured): halves the startup-critical DMA bytes and
runs the QK^T matmuls at full PE rate with fast weight loads.

Device-side plan (per batch, in the TRANSPOSED score layout S^T[k, q]):
    - S^T tile [128k, 512q] = k_slice[d,128k]^T @ qT[d, 512q]  (fp16,
      2 d-half accumulation steps)
    - softmax is shift-invariant: subtract a CONSTANT shift (scores ~ N(0, d)
      with d=256 -> |s| < ~110 always). Mask + shift fold into the ACT-engine
      exp as a per-partition bias: e = exp(s + bias_k), in f32r.
    - out^T[dv, q] += v_tile[128k, dv_chunk]^T @ e   (f32r, full rate)
    - Z: e-tiles are pairwise-accumulated on DVE in groups of QUAD; each
      group closes with a matmul against an all-16s stationary that row-sums
      AND broadcasts 16Z to every partition (folding the post-softmax
      1/sqrt(d)=1/16 scale) into one PSUM bank.
    - out = out^T * (1/(16Z)) via DVE approx-reciprocal + multiply.

DMA schedule (the aggregate HBM bandwidth ~358GB/s is the startup
constraint, so only stripe-0-critical bytes move first):
    - gpsimd SWDGE: q stripe-0 halves + batch-0 k tiles interleaved (the
      first matmul needs q00h0 + k0 only), then the remaining q stripes.
    - sync HWDGE: batch-0 v tiles, then ALL batch-1 tiles (prefetch), then
      o0 stores.
    - scalar HWDGE (= ACT engine): bias only, before the exp stream starts;
      o1 stores ride along later (~0.6us each, inside ACT's slack).
Dep-free fp32 warmup matmuls bridge the PE from ring-open (~7.2us, fixed
NEFF preamble) to the first real matmul so the HAM clock gate reaches
K=8/8 without re-throttling mid-kernel.
"""

from contextlib import ExitStack

import numpy as np

import concourse.bacc as bacc
import concourse.mybir as mybir
import concourse.tile as tile
from concourse.bass_utils import run_bass_kernel_spmd

P = 128          # partitions
D = 256          # d == n_dv
S = 2048         # n_q
NB = 2           # batches per core
QS = 512         # q-stripe width (max fp32 matmul moving dim)
NQS = S // QS    # 4 q-stripes
NCORES = 8
SHIFT = 60.0     # constant softmax shift (see module docstring)

F32 = mybir.dt.float32
F32R = mybir.dt.float32r
F16 = mybir.dt.float16
EXP = mybir.ActivationFunctionType.Exp
MULT = mybir.AluOpType.mult
ADD = mybir.AluOpType.add

N_WARM_MID = 2    # fp32 [128,256] warmups (2 half-rate passes each)
N_WARM_SMALL = 2  # fp32 [128,128] warmups
QUAD = 5          # e-tiles accumulated on DVE per Z matmul


def build(nkt):
    """Build the per-core program. nkt = compacted key length / 128."""
    nc = bacc.Bacc()
    kk = nc.declare_dram_parameter("k", [NB, nkt, P, 2 * P], F16, isOutput=False)
    vv = nc.declare_dram_parameter("v", [NB, nkt, P, 2 * P], F32R, isOutput=False)
    qq = nc.declare_dram_parameter("q", [NB, NQS, P, 2 * QS], F16, isOutput=False)
    bb = nc.declare_dram_parameter("bias", [NB, P, nkt], F32, isOutput=False)
    out = nc.declare_dram_parameter("out", [NB, D, S], F32, isOutput=True)

    with tile.TileContext(nc) as tc, ExitStack() as ctx:
        consts = ctx.enter_context(tc.tile_pool(name="consts", bufs=1))
        inp = ctx.enter_context(tc.tile_pool(name="inp", bufs=1))
        epool = ctx.enter_context(tc.tile_pool(name="e", bufs=4))
        apool = ctx.enter_context(tc.tile_pool(name="a", bufs=2))
        opool = ctx.enter_context(tc.tile_pool(name="o", bufs=2))
        zpool = ctx.enter_context(tc.tile_pool(name="z", bufs=2))
        ps_s = ctx.enter_context(tc.tile_pool(name="ps_s", bufs=3, space="PSUM"))
        ps_o = ctx.enter_context(tc.tile_pool(name="ps_o", bufs=2, space="PSUM"))
        ps_z = ctx.enter_context(tc.tile_pool(name="ps_z", bufs=1, space="PSUM"))

        # ---- constants; memsets on gpsimd (its queue opens first, ~6.0us)
        warm_f = consts.tile([P, P], F32)
        nc.gpsimd.memset(warm_f, 1.0)
        warm_w = consts.tile([P, 2 * P], F32)
        nc.gpsimd.memset(warm_w, 0.5)
        sixteens_f = consts.tile([P, P], F32)
        nc.gpsimd.memset(sixteens_f, 16.0)
        # (memset can't emit f32r; DVE-copy to round)
        sixteens = consts.tile([P, P], F32R)
        nc.vector.tensor_copy(sixteens, sixteens_f)

        # Warmup Exp: attaches the implicit ~2.7us ACT table load to a
        # dep-light instruction so it overlaps the input-DMA fill.
        warm_out = consts.tile([P, 1], F32)
        nc.scalar.activation(warm_out, warm_f[:, 0:1], EXP)

        # PE warmups: dep-free matmuls from ring-open until real matmuls
        # flow, so the HAM clock gate reaches K=8/8 with no re-throttle.
        for w in range(N_WARM_MID):
            wp = ps_s.tile([P, QS], F32, tag="s", name=f"warmb{w}")
            nc.tensor.matmul(
                wp[:, 0 : 2 * P], lhsT=warm_f, rhs=warm_w, start=True, stop=True
            )
        for w in range(N_WARM_SMALL):
            wp = ps_s.tile([P, QS], F32, tag="s", name=f"warms{w}")
            nc.tensor.matmul(
                wp[:, 0:P], lhsT=warm_f, rhs=warm_f[:, 0:P], start=True, stop=True
            )

        # ---- input DMAs, issued upfront in consumption order
        # scalar (ACT) ring: bias only — it must precede the exp stream
        biast = []
        for b in range(NB):
            bt = inp.tile([P, nkt], F32, tag=f"bias{b}")
            nc.scalar.dma_start(out=bt, in_=bb[b])
            biast.append(bt)
        # gpsimd ring: q00 halves + batch-0 k tiles interleaved (the first
        # matmul needs q00h0 + k0 only), then the remaining q stripes
        kts = [[], []]
        q00 = [None, None]
        qts = [[None] * NQS for _ in range(NB)]
        q00[0] = inp.tile([P, QS], F16, tag="q00_0", name="q00_0")
        nc.gpsimd.dma_start(out=q00[0], in_=qq[0, 0, :, 0:QS])
        kt0 = inp.tile([P, 2 * P], F16, tag="k0_0", name="kt0")
        nc.gpsimd.dma_start(out=kt0, in_=kk[0, 0])
        kts[0].append(kt0)
        q00[1] = inp.tile([P, QS], F16, tag="q00_1", name="q00_1")
        nc.gpsimd.dma_start(out=q00[1], in_=qq[0, 0, :, QS : 2 * QS])
        for t in range(1, nkt):
            kt = inp.tile([P, 2 * P], F16, tag=f"k0_{t}")
            nc.gpsimd.dma_start(out=kt, in_=kk[0, t])
            kts[0].append(kt)
        for b in range(NB):
            for s in range(NQS):
                if b == 0 and s == 0:
                    continue
                qt = inp.tile([P, 2 * QS], F16, tag=f"q{b}_{s}")
                nc.gpsimd.dma_start(out=qt, in_=qq[b, s])
                qts[b][s] = qt
        # sync ring: batch-0 v tiles per-tile (streamed under stripe 0),
        # then all of batch 1 in 2 blocks per tensor (prefetch)
        vts = [[], []]
        for t in range(nkt):
            vt = inp.tile([P, 2 * P], F32R, tag=f"v0_{t}")
            nc.sync.dma_start(out=vt, in_=vv[0, t])
            vts[0].append(vt)
        h = (nkt + 1) // 2
        for prefix, param, dt, row in (("k1", kk, F16, kts[1]), ("v1", vv, F32R, vts[1])):
            for t0, t1 in ((0, h), (h, nkt)):
                blk = inp.tile(
                    [P, t1 - t0, 2 * P], dt, tag=f"{prefix}_{t0}", name=f"{prefix}_{t0}"
                )
                nc.sync.dma_start(
                    out=blk, in_=param[1, t0:t1].rearrange("t p c -> p t c")
                )
                row.extend(blk[:, t - t0, :] for t in range(t0, t1))

        # ---- compute, one 512-wide q-stripe at a time
        for b in range(NB):
            for s in range(NQS):
                if b == 0 and s == 0:
                    qh = (q00[0], q00[1])
                else:
                    qt = qts[b][s]
                    qh = (qt[:, 0:QS], qt[:, QS : 2 * QS])
                op0 = ps_o.tile([P, QS], F32, tag="o0", name="op0")
                op1 = ps_o.tile([P, QS], F32, tag="o1", name="op1")
                zp = ps_z.tile([P, QS], F32, tag="z", name="zp")
                nzmm = (nkt + QUAD - 1) // QUAD
                acc, nacc, zi = None, 0, 0
                for t in range(nkt):
                    kt, vt = kts[b][t], vts[b][t]
                    sp = ps_s.tile([P, QS], F32, tag="s", name="sp")
                    nc.tensor.matmul(
                        sp, lhsT=kt[:, 0:P], rhs=qh[0], start=True, stop=False
                    )
                    nc.tensor.matmul(
                        sp, lhsT=kt[:, P : 2 * P], rhs=qh[1], start=False, stop=True
                    )
                    e = epool.tile([P, QS], F32R, tag="e", name="e")
                    nc.scalar.activation(e, sp, EXP, bias=biast[b][:, t : t + 1])
                    first, last = t == 0, t == nkt - 1
                    nc.tensor.matmul(
                        op0, lhsT=vt[:, 0:P], rhs=e, start=first, stop=last
                    )
                    nc.tensor.matmul(
                        op1, lhsT=vt[:, P : 2 * P], rhs=e, start=first, stop=last
                    )
                    # running DVE accumulator for the softmax denominator;
                    # every QUAD tiles one Z matmul folds the partial into
                    # PSUM (row-sum + broadcast of 16Z to every partition,
                    # folding the post-softmax 1/sqrt(d)=1/16 scale)
                    if acc is None:
                        acc, nacc = e, 1
                    else:
                        na = apool.tile([P, QS], F32R, tag="a", name="na")
                        nc.vector.tensor_tensor(na, acc, e, ADD)
                        acc, nacc = na, nacc + 1
                    if nacc == QUAD or t == nkt - 1:
                        nc.tensor.matmul(
                            zp, lhsT=sixteens, rhs=acc,
                            start=zi == 0, stop=zi == nzmm - 1,
                        )
                        zi += 1
                        acc, nacc = None, 0

                # normalize: out = out_unnorm * (1/(16Z)). ~18-bit approx
                # reciprocal; z is far from denorm/inf so edge cases can't
                # hit. Last stripe runs in halves so the tail pipelines.
                halves = 2 if (b == NB - 1 and s == NQS - 1) else 1
                hw = QS // halves
                zbs = zpool.tile([P, QS], F32, tag="zbs", name="zbs")
                o0 = opool.tile([P, QS], F32, tag="so0", name="o0")
                o1 = opool.tile([P, QS], F32, tag="so1", name="o1")
                for h in range(halves):
                    hs = slice(h * hw, (h + 1) * hw)
                    osl = slice(s * QS + h * hw, s * QS + (h + 1) * hw)
                    nc.vector.reciprocal_approx_fast(out=zbs[:, hs], in_=zp[:, hs])
                    nc.vector.tensor_tensor(o0[:, hs], op0[:, hs], zbs[:, hs], MULT)
                    nc.vector.tensor_tensor(o1[:, hs], op1[:, hs], zbs[:, hs], MULT)
                    nc.sync.dma_start(out=out[b, 0:P, osl], in_=o0[:, hs])
                    nc.scalar.dma_start(out=out[b, P : 2 * P, osl], in_=o1[:, hs])

    return nc


def make_in_maps(q, k, v, mask):
    """Shard over batch; tile + compact; cast q,k to fp16."""
    q = np.asarray(q, dtype=np.float32)
    k = np.asarray(k, dtype=np.float32)
    v = np.asarray(v, dtype=np.float32)
    mask = np.asarray(mask, dtype=np.int32).reshape(len(q), -1)

    B = len(q)
    idxs = [np.nonzero(mask[b])[0] for b in range(B)]
    n_eff = max((len(ix) for ix in idxs), default=1)
    sk = max(P, ((n_eff + P - 1) // P) * P)  # padded compacted key length
    nkt = sk // P

    kg = np.zeros((B, D, sk), dtype=np.float32)
    vg = np.zeros((B, sk, D), dtype=np.float32)
    # exp bias: -SHIFT for real keys, -1e9 for padding (kills it exactly)
    bg = np.full((B, sk), -1.0e9, dtype=np.float32)
    for b in range(B):
        ix = idxs[b]
        kg[b, :, : len(ix)] = k[b][:, ix]
        vg[b, : len(ix)] = v[b][ix]
        bg[b, : len(ix)] = -SHIFT

    # k tiles: [B, nkt, P, 256] fp16: cols 0:128 = d-half0 keys, 128:256 = half1
    kh = np.ascontiguousarray(
        kg.reshape(B, 2, P, nkt, P).transpose(0, 3, 2, 1, 4).reshape(B, nkt, P, 2 * P)
    ).astype(np.float16)
    # v tiles: [B, nkt, P, 256] f32: partition = key-within-tile, cols = dv
    vh = np.ascontiguousarray(vg.reshape(B, nkt, P, 2 * P))
    # q stripes: [B, NQS, P, 1024] fp16 = both d-halves of qT side by side
    qt = np.transpose(q, (0, 2, 1))  # [B, D, S]
    qh = np.ascontiguousarray(
        qt.reshape(B, 2, P, NQS, QS).transpose(0, 3, 2, 1, 4).reshape(B, NQS, P, 2 * QS)
    ).astype(np.float16)
    bgt = np.ascontiguousarray(bg.reshape(B, nkt, P).transpose(0, 2, 1))  # [B, P, nkt]

    in_maps = []
    for i in range(NCORES):
        sl = slice(i * NB, (i + 1) * NB)
        in_maps.append(
            {
                "k": np.ascontiguousarray(kh[sl]),
                "v": np.ascontiguousarray(vh[sl]),
                "q": np.ascontiguousarray(qh[sl]),
                "bias": np.ascontiguousarray(bgt[sl]),
            }
        )
    return in_maps, nkt


def run(q, k, v, mask, **kwargs):
    in_maps, nkt = make_in_maps(q, k, v, mask)
    nc = build(nkt)
    nc.finalize()  # run the Bacc pass pipeline (reg alloc, wait splitting)
    res = run_bass_kernel_spmd(nc, in_maps, list(range(NCORES)), **kwargs)
    out = np.concatenate([r["out"] for r in res.results], axis=0)
    return out, res


def kernel(q, k, v, mask):
    out, _ = run(q, k, v, mask)
    return out
